# revision 9
# baseline (speedup 1.0000x reference)
"""CARAFE kernel for 8x Trainium2 NeuronCores — PE-based reassembly.

Core = b*4 + q (batch, H-quarter). Per core:
  comp 1x1 conv + enc 3x3 conv (bf16) as in the baseline.
  exp(enc) per (row-pair lam, subpixel dd) with accum_out = softmax sums.
  The exp'd weights are DMA-scattered into a DRAM scratch laid out
  [ki][jt_guarded][out_pixrow][j][dd] so the banded reassembly matrices
  B[src pixel, out pixel] read back as pure rectangles (zero cells come
  from the host-zeroed scratch; guard slots absorb image-edge bands).
  Reassembly: per (lam, dd) out tile, 3 accumulating PE matmuls
  out[(rho,j), c] += B_role^T @ x_rowpair, one per source row-pair role.
  Softmax normalization is the PSUM->SBUF eviction scale (per-partition).
"""

import sys

if "/opt/trn_rl_repo" not in sys.path:
    sys.path.insert(0, "/opt/trn_rl_repo")

import numpy as np
import ml_dtypes

BF16 = ml_dtypes.bfloat16

SCALE = 2
K_UP = 5
EPS = 1e-5
B, C, H, W = 2, 256, 64, 64
CM = 64
NK = 100
NCORES = 8
QH = 16
NW1R = 18

# scratch layout strides (elements): addr = ki*S_A + jt*S_JT + pr*S_PR + j*4 + dd
# with jt = j + kj (2 guard slots each side). S_PR = 64*(S_JT+4) makes the
# write's 128 (rho,j) partitions a single uniform-stride dim; band spill
# past S_PR lands only in the next pr's never-read guard slots.
S_DD = 1
S_J = 4
S_JT = 512
S_PR = 64 * (S_JT + S_J)     # 33024
S_A = 532480
SCR_N = 5 * S_A              # 2662400
JT_GUARD = 2

_compiled = {}


def _build_nc():
    import concourse.bacc as bacc
    import concourse.bass as bass
    import concourse.mybir as mybir
    import concourse.tile as tile

    f32 = mybir.dt.float32
    bf16 = mybir.dt.bfloat16
    nc = bacc.Bacc("TRN2", target_bir_lowering=False, debug=False)

    x_nat_d = nc.dram_tensor("x_nat", [128, 2, NW1R, W], bf16, kind="ExternalInput")
    x_pix_d = nc.dram_tensor("x_pix", [128, 10, C], bf16, kind="ExternalInput")
    comp_lhsT_d = nc.dram_tensor("comp_lhsT", [128, 2, CM], bf16, kind="ExternalInput")
    comp_bias_d = nc.dram_tensor("comp_bias", [CM, 1], f32, kind="ExternalInput")
    enc_pair_d = nc.dram_tensor("enc_pair", [128, 3, NK], bf16, kind="ExternalInput")
    enc_single_d = nc.dram_tensor("enc_single", [CM, 3, NK], bf16, kind="ExternalInput")
    enc_bias_d = nc.dram_tensor("enc_bias", [1, NK], bf16, kind="ExternalInput")
    rmask_d = nc.dram_tensor("rmask", [CM, NW1R], bf16, kind="ExternalInput")
    scr_ds = [
        nc.dram_tensor(f"scr{i}", [1, SCR_N], bf16, kind="ExternalOutput")
        for i in range(4)
    ]
    out_d = nc.dram_tensor("out", [8, 128, 4, C], bf16, kind="ExternalOutput")

    def scr_ap(which, off_els, dims):
        sl = scr_ds[which][0:1, off_els : off_els + 1]
        return bass.AP(
            tensor=sl.tensor,
            offset=sl.offset,
            ap=[[1, 1]] + [list(d) for d in dims],
        )

    with tile.TileContext(nc) as tc:
        with (
            tc.tile_pool(name="consts", bufs=1) as consts,
            tc.tile_pool(name="big", bufs=1) as big,
            tc.tile_pool(name="epool", bufs=3) as epool,
            tc.tile_pool(name="bpool", bufs=1) as bpool,
            tc.tile_pool(name="opool", bufs=4) as opool,
            tc.tile_pool(name="pcomp", bufs=1, space="PSUM") as pcomp,
            tc.tile_pool(name="penc", bufs=2, space="PSUM") as penc,
            tc.tile_pool(name="pout", bufs=3, space="PSUM") as pout,
        ):
            # ---- constants ----
            comp_lhsT = consts.tile([128, 2, CM], bf16, tag="comp_lhsT")
            nc.scalar.dma_start(comp_lhsT, comp_lhsT_d[:])
            comp_bias = consts.tile([CM, 1], f32, tag="comp_bias")
            nc.scalar.dma_start(comp_bias, comp_bias_d[:])
            enc_pair = consts.tile([128, 3, NK], bf16, tag="enc_pair")
            nc.scalar.dma_start(enc_pair, enc_pair_d[:])
            enc_single = consts.tile([CM, 3, NK], bf16, tag="enc_single")
            nc.scalar.dma_start(enc_single, enc_single_d[:])
            enc_bias = consts.tile([1, NK], bf16, tag="enc_bias")
            nc.scalar.dma_start(enc_bias, enc_bias_d[:])
            rmask = consts.tile([CM, NW1R], bf16, tag="rmask")
            nc.scalar.dma_start(rmask, rmask_d[:])
            ones_row = consts.tile([1, 128], bf16, tag="ones_row")
            nc.vector.memset(ones_row, 1.0)

            # ---- x ----
            x_nat = big.tile([128, 2, NW1R, W], bf16, tag="x_nat")
            nc.scalar.dma_start(x_nat, x_nat_d[:])
            x_pix = big.tile([128, 10, C], bf16, tag="x_pix")
            nc.scalar.dma_start(x_pix, x_pix_d[:])

            # ---- B tiles: [128 (rt, jt), rho 2, j 64, dd 4] per role/phase,
            # zeroed once; invalid quadrants + off-band cells stay zero ----
            btiles = {}
            for ph in range(3):
                t = bpool.tile([128, 3, 2, W, 4], bf16, tag=f"B_{ph}")
                nc.vector.memset(t, 0.0)
                btiles[ph] = t

            # ---- comp 1x1 conv -> W1 ----
            psum_c = pcomp.tile([CM, NW1R * W], f32, tag="psum_c")
            nchunks = [(0, 512), (512, 512), (1024, NW1R * W - 1024)]
            x_nat_f = x_nat.rearrange("p h r w -> p h (r w)")
            for h in range(2):
                for ci, (n0, nl) in enumerate(nchunks):
                    nc.tensor.matmul(
                        psum_c[:, n0 : n0 + nl],
                        lhsT=comp_lhsT[:, h, :],
                        rhs=x_nat_f[:, h, n0 : n0 + nl],
                        start=(h == 0),
                        stop=(h == 1 and ci == len(nchunks) - 1),
                    )

            w1c = big.tile([CM, NW1R, W], bf16, tag="w1c")
            psum_c_v = psum_c.rearrange("p (r w) -> p r w", w=W)
            nc.scalar.activation(
                out=w1c,
                in_=psum_c_v,
                func=mybir.ActivationFunctionType.Relu,
                bias=comp_bias,
                scale=1.0,
            )
            rmask_b = bass.AP(
                tensor=rmask.tensor,
                offset=rmask.offset,
                ap=[list(rmask.ap[0]), list(rmask.ap[1]), [0, W]],
            )
            nc.vector.tensor_tensor(
                out=w1c, in0=w1c, in1=rmask_b, op=mybir.AluOpType.mult
            )
            w1main = big.tile([128, NW1R, W], bf16, tag="w1main")
            nc.vector.memset(w1main, 0.0)
            nc.vector.tensor_copy(out=w1main[CM:128], in_=w1c)
            nc.vector.tensor_copy(out=w1main[0:CM, :, 1:W], in_=w1c[:, :, 0 : W - 1])
            w1sing = big.tile([CM, NW1R, W], bf16, tag="w1sing")
            nc.vector.memset(w1sing, 0.0)
            nc.vector.tensor_copy(out=w1sing[:, :, 0 : W - 1], in_=w1c[:, :, 1:W])
            w1main_f = w1main.rearrange("p r w -> p (r w)")
            w1sing_f = w1sing.rearrange("p r w -> p (r w)")

            S = big.tile([128, 8, 4], f32, tag="S")
            S_f = S.rearrange("p l q -> p (l q)")
            R = big.tile([128, 8, 4], f32, tag="R")
            R_f = R.rearrange("p l q -> p (l q)")

            LAM_ENG = {0: 0, 1: 1, 2: 0, 3: 0, 4: 1, 5: 0, 6: 0, 7: 1}
            LAM_SCR = {}
            _cnt = [0, 0]
            for _l in range(8):
                _e = LAM_ENG[_l]
                LAM_SCR[_l] = _e + 2 * (_cnt[_e] % 2)
                _cnt[_e] += 1

            for lam in range(8):
                # ---- enc conv for row pair lam; psum partitions = (rho, j) ----
                psum_e = penc.tile([128, NK], f32, tag="psum_e")
                for ty in range(3):
                    o = (2 * lam + ty) * W
                    nc.tensor.matmul(
                        psum_e,
                        lhsT=w1main_f[:, o : o + 2 * W],
                        rhs=enc_pair[:, ty, :],
                        start=(ty == 0),
                        stop=False,
                    )
                for ty in range(3):
                    o = (2 * lam + ty) * W
                    nc.tensor.matmul(
                        psum_e,
                        lhsT=w1sing_f[:, o : o + 2 * W],
                        rhs=enc_single[:, ty, :],
                        start=False,
                        stop=False,
                    )
                nc.tensor.matmul(
                    psum_e, lhsT=ones_row, rhs=enc_bias, start=False, stop=True
                )

                # ---- exp + per-dd sums; ch = (ki*5+kj)*4 + dd ----
                E = epool.tile([128, NK], bf16, tag="E")
                psum_e_v = psum_e.rearrange("p (t d) -> p d t", d=4)
                E_v = E.rearrange("p (t d) -> p d t", d=4)
                for dd in range(4):
                    idx = lam * 4 + dd
                    nc.scalar.activation(
                        out=E_v[:, dd, :],
                        in_=psum_e_v[:, dd, :],
                        func=mybir.ActivationFunctionType.Exp,
                        accum_out=S_f[:, idx : idx + 1],
                    )
                nc.vector.reciprocal(
                    out=R_f[:, lam * 4 : lam * 4 + 4],
                    in_=S_f[:, lam * 4 : lam * 4 + 4],
                )

                # ---- scatter exp'd weights into scratch (one DMA per kj) ----
                # src enumerates (rho 2, j 64 | ki 5, dd 4); dst walks
                # (pixrow, jt&j diagonal, ki, dd) with jt = j + kj (guarded).
                E5 = E.rearrange("p (a b d) -> p a b d", b=5, d=4)
                deng = nc.sync if LAM_ENG[lam] == 0 else nc.gpsimd
                for kj in range(K_UP):
                    wsrc = E5[:, :, kj, :]  # [128, 5, 4]; partitions (rho,j)
                    dst = scr_ap(
                        LAM_SCR[lam],
                        (2 * lam) * S_PR + kj * S_JT,
                        [
                            [S_JT + S_J, 128],
                            [S_A, 5],
                            [S_DD, 4],
                        ],
                    )
                    deng.dma_start(dst, wsrc)

                # drain this lam's DMA engine: block until the scatter
                # writes (and any older outstanding DMAs on it) complete
                deng.drain()

                # ---- load B tiles: one DMA per (rt, rho) covering the
                # contiguous valid role range (role stride = 2*S_A) ----
                ph = lam % 3
                bt = btiles[ph]
                for rt in range(2):
                    for rho in range(2):
                        roles = [
                            r for r in (-1, 0, 1)
                            if 0 <= 2 * r + rt - rho + 2 < K_UP
                        ]
                        r0 = roles[0]
                        a0 = 2 * r0 + rt - rho + 2
                        dst = bt[64 * rt : 64 * rt + 64, r0 + 1 : roles[-1] + 2, rho]
                        src = scr_ap(
                            LAM_SCR[lam],
                            a0 * S_A
                            + JT_GUARD * S_JT
                            + (2 * lam + rho) * S_PR,
                            [
                                [S_JT, 64],
                                [2 * S_A, len(roles)],
                                [S_DD, W * 4],
                            ],
                        )
                        deng.dma_start(dst, src)

                # ---- reassembly matmuls + normalized eviction ----
                ot = opool.tile([128, 4, C], bf16, tag="ot")
                for dd in range(4):
                    po = pout.tile([128, C], f32, tag="po")
                    for role in (-1, 0, 1):
                        lhsT = bt[:, role + 1, :, :, dd].rearrange(
                            "p r j -> p (r j)"
                        )
                        nc.tensor.matmul(
                            po,
                            lhsT=lhsT,
                            rhs=x_pix[:, lam + role + 1, :],
                            start=(role == -1),
                            stop=(role == 1),
                        )
                    idx = lam * 4 + dd
                    nc.scalar.activation(
                        out=ot[:, dd, :],
                        in_=po,
                        func=mybir.ActivationFunctionType.Copy,
                        scale=R_f[:, idx : idx + 1],
                    )
                nc.scalar.dma_start(out_d[lam], ot)

    nc.compile()
    return nc


def _host_inputs(x, comp_w, comp_gamma, comp_beta, comp_mean, comp_var,
                 enc_w, enc_gamma, enc_beta, enc_mean, enc_var):
    x = np.asarray(x, dtype=np.float32)
    scale_c = (np.asarray(comp_gamma) / np.sqrt(np.asarray(comp_var) + EPS)).astype(
        np.float32
    )
    bias_c = (np.asarray(comp_beta) - np.asarray(comp_mean) * scale_c).astype(
        np.float32
    )
    wp = np.asarray(comp_w)[:, :, 0, 0].astype(np.float32) * scale_c[:, None]
    comp_lhsT = np.ascontiguousarray(
        wp.T.reshape(2, 128, CM).transpose(1, 0, 2)
    ).astype(BF16)
    comp_bias = bias_c.reshape(CM, 1)

    scale_e = (np.asarray(enc_gamma) / np.sqrt(np.asarray(enc_var) + EPS)).astype(
        np.float32
    )
    bias_e = (np.asarray(enc_beta) - np.asarray(enc_mean) * scale_e).astype(np.float32)
    ew = np.asarray(enc_w).astype(np.float32) * scale_e[:, None, None, None]
    enc_pair = np.zeros((128, 3, NK), np.float32)
    enc_single = np.zeros((CM, 3, NK), np.float32)
    for ty in range(3):
        enc_pair[0:CM, ty] = ew[:, :, ty, 0].T
        enc_pair[CM:128, ty] = ew[:, :, ty, 1].T
        enc_single[:, ty] = ew[:, :, ty, 2].T
    enc_pair = enc_pair.astype(BF16)
    enc_single = enc_single.astype(BF16)
    enc_bias_a = bias_e.reshape(1, NK).astype(BF16)

    in_maps = []
    for core in range(NCORES):
        b, q = core // 4, core % 4
        # x_nat: rows [16q-1, 16q+17) cols padded, ch-partition layout
        xpad = np.zeros((C, 21, W + 4), np.float32)
        g0, g1 = 16 * q - 2, 16 * q + 19
        s0, s1 = max(g0, 0), min(g1, H)
        xpad[:, s0 - g0 : s1 - g0, 2 : W + 2] = x[b, :, s0:s1, :]
        x_nat = np.ascontiguousarray(
            xpad[:, 1 : 1 + NW1R, 2 : W + 2].reshape(2, 128, NW1R, W).transpose(
                1, 0, 2, 3
            )
        ).astype(BF16)

        # x_pix: partition (rho, j) rho-major; slab s = local rows 2s-2, 2s-1
        xs = np.zeros((128, 10, C), np.float32)
        for s in range(10):
            for rho in range(2):
                r = 16 * q - 2 + 2 * s + rho
                if 0 <= r < H:
                    # partitions rho*64 + j, channels in free dim
                    xs[rho * 64 : rho * 64 + 64, s, :] = x[b, :, r, :].T
        x_pix = xs.astype(BF16)

        ridx = np.arange(NW1R)
        grows = 16 * q - 1 + ridx
        rmask = np.ascontiguousarray(
            np.broadcast_to(
                ((grows >= 0) & (grows < H)).astype(np.float32), (CM, NW1R)
            )
        ).astype(BF16)

        m = {
            "x_nat": x_nat,
            "x_pix": x_pix,
            "comp_lhsT": comp_lhsT,
            "comp_bias": comp_bias,
            "enc_pair": enc_pair,
            "enc_single": enc_single,
            "enc_bias": enc_bias_a,
            "rmask": rmask,
        }
        in_maps.append(m)
    return in_maps


def _assemble(results):
    out = np.zeros((B, C, H * SCALE, W * SCALE), np.float32)
    for core in range(NCORES):
        b, q = core // 4, core % 4
        arr = results[core]["out"].astype(np.float32)  # [8, 128, 4, C]
        # out pixel: row 32q + 4*lam + 2*rho + di, col 2*j + dj
        a = arr.reshape(8, 2, 64, 2, 2, C)  # [lam, rho, j, di, dj, c]
        blk = a.transpose(5, 0, 1, 3, 2, 4).reshape(C, 32, 128)
        out[b, :, 32 * q : 32 * q + 32, :] = blk
    return out


def kernel(**inputs):
    from concourse.bass_utils import run_bass_kernel_spmd

    if "nc" not in _compiled:
        _compiled["nc"] = _build_nc()
    nc = _compiled["nc"]
    in_maps = _host_inputs(**inputs)
    res = run_bass_kernel_spmd(nc, in_maps, core_ids=list(range(NCORES)))
    return _assemble(res.results)
